# revision 26
# baseline (speedup 1.0000x reference)
"""FFM layer kernel for 8 Trainium2 NeuronCores — fp8 DoubleRow edition.

Math (reference): x[B,39] = 13 dense cols + 26 sparse index cols (ints 0..99
stored as f32).  inputs[B,2613] = [dense | one_hot(sparse)], then
  linear = inputs @ w.T + b
  field  = einsum('bn,nfk->bfk', inputs, v)        # [B,39,16]
  cross  = 0.5*sum_k((sum_f field)^2 - sum_f field^2)
  out    = sigmoid(linear + cross)

Strategy: data-parallel over batch, 2048 rows/core.  The sparse part is an
embedding-bag done as a one-hot matmul; the one-hot lhs is exact in fp8, so
the sparse contraction runs as fp8e4 MatmulPerfMode.DoubleRow (two 128-row
k-chunks per instruction; measured 1 cycle per output column per 256-row
pair = 2x fp16).  PSUM columns (658):
  [0:624)   field, k-major f-minor, v quantized fp8 at scale 1024 (sq only)
  [624:641) hi block: 16 U_hi cols (U = sum_f v, scale 512) + w_hi col
            (scale 1024, bias folded into the dense matmul)
  [641:658) lo block: fp8 quantization residuals at 16x the hi scale
s and linear are reconstructed as hi + lo/16, which recovers ~fp16 accuracy
for the dominant (sum_f field)^2 term; the field cols feed only sum field^2
where fp8 error is quadratically suppressed.  The first 2560 one-hot rows
fill exactly 10 DoubleRow pairs; the last 40 one-hot rows ride in the fp16
dense matmul (13 dense features + bias row + 40 one-hot rows; fp16 cost is
per-column so the extra rows are free) — this saves a whole 11th pair.

The fp8 one-hot matrix is built on the HOST and DMAed directly: at 1 byte
per cell it is the same traffic as shipping replicated indices for an
on-device is_equal build (the fp16-era tradeoff), and it frees the vector
engine for the epilogue.  One DMA per 128-row batch tile (oh_d laid out
[part, tile, chunk, col] so each transfer is one 2.5KB contiguous run per
partition) gives the PE per-tile granularity: the first matmuls start as
soon as the first 320KB tile + the first vperm pairs land.  vperm arrives
pair-by-pair for the same reason.  A short burst of warmup matmuls starts
the HAM clock ramp (cold PE runs at ~1.2GHz; full speed only arrives after
~20us of sustained activity, so the early tiles are slower regardless).
"""

import sys

sys.path.insert(0, "/opt/trn_rl_repo")

import numpy as np
import ml_dtypes

import concourse.tile as tile
from concourse import bacc, mybir
from concourse.bass_utils import run_bass_kernel_spmd

N_CORES = 8
B_FULL = 16384
BC = B_FULL // N_CORES  # 2048 rows per core
P = 128
N_DENSE = 13
N_SPARSE = 26
SPARSE_DIM = 100
N_FIELD = 39
K_DIM = 16
NOH = 20                # one-hot chunks of 128 rows in fp8 (rows 0..2559)
NPAIR = 10
NSPROWS = N_SPARSE * SPARSE_DIM  # 2600
NTAIL = NSPROWS - NOH * P        # 40 rows folded into the dense matmul
# dense-matmul stationary layout: rows 0..39 = one-hot tail, 40..63 = zeros,
# 64 = bias carrier, 65..77 = x_dense
DN0 = 64                         # first dense row
NDN = DN0 + 1 + N_DENSE          # 78 contraction rows
COLS = 658              # 624 field | 16 U_hi + w_hi | 16 U_lo + w_lo
FLD = N_FIELD * K_DIM   # 624
HI0 = FLD               # 624
LO0 = FLD + K_DIM + 1   # 641

SC_V = 1024.0
SC_U = 512.0
SC_W = 1024.0
LOGAIN = 16.0           # lo cols stored at hi_scale * LOGAIN

F16 = mybir.dt.float16
F32 = mybir.dt.float32
F8 = mybir.dt.float8e4
I8 = mybir.dt.int8

_prog_cache = {}


def _build_program(bc):
    """One SPMD program for a batch slice of `bc` rows (all cores identical)."""
    nbt = bc // P

    nc = bacc.Bacc("TRN2", target_bir_lowering=False, debug=False)
    # all dram tensors are partition-major on the host so every DMA moves
    # long contiguous runs per partition
    oh_d = nc.declare_dram_parameter(
        "oh", [P, nbt, NOH, P], F8, isOutput=False)
    xdn_d = nc.declare_dram_parameter("xdn", [NDN, bc], F16, isOutput=False)
    vd_d = nc.declare_dram_parameter("vdense", [NDN, COLS], F16,
                                     isOutput=False)
    vp_d = nc.declare_dram_parameter(
        "vperm", [P, NPAIR, 2, COLS], F8, isOutput=False)
    y_d = nc.declare_dram_parameter("y", [P, nbt], F32, isOutput=True)

    with tile.TileContext(nc) as tc:
        with (
            tc.tile_pool(name="pers", bufs=1) as pers,
            tc.tile_pool(name="psum", bufs=4, space="PSUM") as psum,
            tc.tile_pool(name="epi", bufs=3) as epi,
        ):
            oh_all = pers.tile([P, nbt, NOH, P], F8, tag="oh", name="oh")
            y_all = pers.tile([P, nbt], F32, tag="yall")
            vp_all = pers.tile([P, NPAIR, 2, COLS], F8, tag="vp")
            vd_t = pers.tile([NDN, COLS], F16, tag="vd")
            xdn_t = pers.tile([NDN, bc], F16, tag="xdn")

            # DMA plan (the head is HBM-bound: the first tile's one-hot +
            # all vperm pairs must land before tile 0 can finish):
            #   sync/scalar: alternating per-tile one-hot slices
            #   gpsimd:      warmup memsets, then vperm pair-by-pair, vdense
            #   scalar:      xdn (needed by the dense matmul, last per tile)
            wz8 = pers.tile([P, 2, P], F8, tag="wz8")
            wz8b = pers.tile([P, 2, 512], F8, tag="wz8b")
            nc.gpsimd.memset(wz8[:], 0.0)
            nc.gpsimd.memset(wz8b[:], 0.0)
            # xdn/vd lead their queues: tile 0's dense matmul (which closes
            # its PSUM accumulation group) needs them, and with 4 PSUM bufs
            # a late tile-0 close stalls the whole pipeline
            nc.scalar.dma_start(xdn_t[:, 0:512], xdn_d[:, 0:512])
            nc.gpsimd.dma_start(vd_t[:], vd_d[:])
            for bt in range(nbt):
                eng = nc.sync if bt % 2 == 0 else nc.scalar
                eng.dma_start(oh_all[:, bt, :, :], oh_d[:, bt, :, :])
            for pl, ph in ((0, 1), (1, 2), (2, 4), (4, 6), (6, 8),
                           (8, NPAIR)):
                nc.gpsimd.dma_start(vp_all[:, pl:ph, :, :],
                                    vp_d[:, pl:ph, :, :])
            nc.scalar.dma_start(xdn_t[:, 512:bc], xdn_d[:, 512:bc])

            # PE warmup: throwaway matmuls during the DMA head start the HAM
            # clock ramp before the first real matmuls are ready
            wps = psum.tile([P, COLS], F32, tag="ps", name="warmps")
            for _ in range(28):
                nc.tensor.matmul(wps[:, 0:512], wz8[:], wz8b[:],
                                 start=True, stop=True,
                                 perf_mode=mybir.MatmulPerfMode.DoubleRow)

            for bt in range(nbt):
                bs = slice(bt * P, (bt + 1) * P)
                ps = psum.tile([P, COLS], F32, tag="ps")
                # sparse one-hot contraction, fp8 DoubleRow
                for p_ in range(NPAIR):
                    lhs = oh_all[:, bt, 2 * p_:2 * p_ + 2, :]
                    nc.tensor.matmul(
                        ps[:, 0:512], lhs, vp_all[:, p_, :, 0:512],
                        start=(p_ == 0), stop=False,
                        perf_mode=mybir.MatmulPerfMode.DoubleRow,
                    )
                    nc.tensor.matmul(
                        ps[:, 512:COLS], lhs, vp_all[:, p_, :, 512:COLS],
                        start=(p_ == 0), stop=False,
                        perf_mode=mybir.MatmulPerfMode.DoubleRow,
                    )
                # dense + bias + one-hot tail rows, fp16 (DoublePixel was
                # probed here and is a silent no-op on TRN2)
                nc.tensor.matmul(ps[:, 0:512], xdn_t[:, bs],
                                 vd_t[:, 0:512], start=False, stop=True)
                nc.tensor.matmul(ps[:, 512:COLS], xdn_t[:, bs],
                                 vd_t[:, 512:COLS], start=False, stop=True)
                # epilogue: s/lin = hi + lo/16; sq = sum field^2;
                # cross + sigmoid with all scale factors folded in
                t1 = epi.tile([P, K_DIM + 1], F32, tag="t1")
                nc.vector.tensor_scalar(
                    out=t1[:], in0=ps[:, LO0:COLS],
                    scalar1=1.0 / LOGAIN, scalar2=None,
                    op0=mybir.AluOpType.mult,
                )
                slin = epi.tile([P, K_DIM + 1], F32, tag="slin")
                nc.vector.tensor_tensor(
                    out=slin[:], in0=t1[:], in1=ps[:, HI0:LO0],
                    op=mybir.AluOpType.add,
                )
                if bt == nbt - 1:
                    # issue the DVE half of sq FIRST so it runs parallel
                    # with the Scalar half instead of after the s/lin chain
                    SQH = 384
                    sq_cp = epi.tile([P, FLD - SQH], F32, tag="sqcp")
                    nc.vector.tensor_copy(sq_cp[:], ps[:, SQH:FLD])
                    sq_scr2 = epi.tile([P, FLD - SQH], F32, tag="sqscr2")
                    sqsumB = epi.tile([P, 1], F32, tag="sqsumB")
                    nc.vector.scalar_tensor_tensor(
                        out=sq_scr2[:], in0=sq_cp[:],
                        scalar=1.0, in1=sq_cp[:],
                        op0=mybir.AluOpType.mult,
                        op1=mybir.AluOpType.mult,
                        accum_out=sqsumB[:],
                    )
                if bt < nbt - 1:
                    sq_scr = epi.tile([P, FLD], F32, tag="sqscr")
                    sqsum = epi.tile([P, 1], F32, tag="sqsum")
                    nc.scalar.activation(
                        out=sq_scr[:], in_=ps[:, 0:FLD],
                        func=mybir.ActivationFunctionType.Square,
                        accum_out=sqsum[:],
                    )
                else:
                    # last tile: the sq reduction is on the serial tail
                    # chain — the DVE half was issued above; Scalar does
                    # the first 384 columns in parallel
                    sq_scr = epi.tile([P, SQH], F32, tag="sqscr")
                    sqsumA = epi.tile([P, 1], F32, tag="sqsumA")
                    nc.scalar.activation(
                        out=sq_scr[:], in_=ps[:, 0:SQH],
                        func=mybir.ActivationFunctionType.Square,
                        accum_out=sqsumA[:],
                    )
                    sqsum = epi.tile([P, 1], F32, tag="sqsum")
                    nc.vector.tensor_scalar(
                        out=sqsum[:], in0=sqsumA[:],
                        scalar1=sqsumB[:], scalar2=None,
                        op0=mybir.AluOpType.add,
                    )
                s2_scr = epi.tile([P, K_DIM], F32, tag="s2scr")
                s2sum = epi.tile([P, 1], F32, tag="s2sum")
                nc.vector.scalar_tensor_tensor(
                    out=s2_scr[:], in0=slin[:, 0:K_DIM],
                    scalar=1.0, in1=slin[:, 0:K_DIM],
                    op0=mybir.AluOpType.mult,
                    op1=mybir.AluOpType.mult,
                    accum_out=s2sum[:],
                )
                # b2 = lin - 0.5*sq (true units) off the critical path
                b2a = epi.tile([P, 1], F32, tag="b2a")
                nc.vector.tensor_scalar(
                    out=b2a[:], in0=slin[:, K_DIM:K_DIM + 1],
                    scalar1=1.0 / SC_W, scalar2=None,
                    op0=mybir.AluOpType.mult,
                )
                b2 = epi.tile([P, 1], F32, tag="b2")
                nc.vector.tensor_scalar(
                    out=b2[:], in0=sqsum[:],
                    scalar1=-0.5 / (SC_V * SC_V), scalar2=b2a[:],
                    op0=mybir.AluOpType.mult,
                    op1=mybir.AluOpType.add,
                )
                nc.scalar.activation(
                    out=y_all[:, bt:bt + 1], in_=s2sum[:],
                    func=mybir.ActivationFunctionType.Sigmoid,
                    scale=0.5 / (SC_U * SC_U), bias=b2[:],
                )
            nc.sync.dma_start(y_d[:], y_all[:])

    nc.compile()
    return nc


def _get_program(bc):
    if bc not in _prog_cache:
        _prog_cache[bc] = _build_program(bc)
    return _prog_cache[bc]


def _q8(a, scale):
    return np.clip(a * scale, -240.0, 240.0).astype(ml_dtypes.float8_e4m3)


def _prep_shared(w_weight, w_bias, v):
    """vperm fp8 [128, 10, 2, 658], vdense fp16 [78, 658]."""
    # field col j = k*39 + f  <->  v[n, f, k]
    v2 = np.ascontiguousarray(v.transpose(0, 2, 1)).reshape(2613, FLD)
    u = v.sum(axis=1)                       # [2613, 16]
    w = w_weight[0]                         # [2613]

    # sparse rows 0..2559 -> fp8 hi (+ lo residual for U and w)
    ns = NOH * P
    rows = np.zeros((ns, COLS), ml_dtypes.float8_e4m3)
    sp = slice(N_DENSE, N_DENSE + ns)
    uhi = _q8(u[sp], SC_U)
    ulo = _q8(u[sp] - uhi.astype(np.float32) / SC_U, SC_U * LOGAIN)
    whi = _q8(w[sp], SC_W)
    wlo = _q8(w[sp] - whi.astype(np.float32) / SC_W, SC_W * LOGAIN)
    rows[:, 0:FLD] = _q8(v2[sp], SC_V)
    rows[:, HI0:HI0 + K_DIM] = uhi
    rows[:, HI0 + K_DIM] = whi
    rows[:, LO0:LO0 + K_DIM] = ulo
    rows[:, LO0 + K_DIM] = wlo
    # device layout [part, pair, j, col]; sparse row = (pair*2+j)*128 + part
    vperm = np.ascontiguousarray(
        rows.reshape(NPAIR, 2, P, COLS).transpose(2, 0, 1, 3))

    # dense rows at the same column scales as the fp8 side but in fp16
    tl = slice(N_DENSE + ns, 2613)
    vdense = np.zeros((NDN, COLS), np.float32)
    vdense[0:NTAIL, 0:FLD] = v2[tl] * SC_V
    vdense[0:NTAIL, HI0:HI0 + K_DIM] = u[tl] * SC_U
    vdense[0:NTAIL, HI0 + K_DIM] = w[tl] * SC_W
    vdense[DN0, HI0 + K_DIM] = float(w_bias[0]) * SC_W
    vdense[DN0 + 1:, 0:FLD] = v2[:N_DENSE] * SC_V
    vdense[DN0 + 1:, HI0:HI0 + K_DIM] = u[:N_DENSE] * SC_U
    vdense[DN0 + 1:, HI0 + K_DIM] = w[:N_DENSE] * SC_W
    return vperm, vdense.astype(np.float16)


def _prep_core(x_core):
    """Host-built fp8 one-hot [part, tile, chunk, col] and xdn fp16."""
    bc = x_core.shape[0]
    idx = x_core[:, N_DENSE:].astype(np.int32)          # [bc, 26]
    flat = idx + np.arange(N_SPARSE)[None, :] * SPARSE_DIM
    # rows 0..2559 in fp8 one-hot; rows 2560..2599 in the fp16 dense tile
    oh = np.zeros((NOH * P, bc), ml_dtypes.float8_e4m3)
    one = ml_dtypes.float8_e4m3(1.0)
    b = np.arange(bc)
    for s in range(N_SPARSE):
        r = flat[:, s]
        m = r < NOH * P
        oh[r[m], b[m]] = one
    ohdev = np.ascontiguousarray(
        oh.reshape(NOH, P, bc // P, P).transpose(1, 2, 0, 3))

    xdn = np.zeros((NDN, bc), np.float16)
    for s in range(N_SPARSE):
        r = flat[:, s]
        m = r >= NOH * P
        xdn[r[m] - NOH * P, b[m]] = 1.0
    xdn[DN0] = 1.0
    xdn[DN0 + 1:] = x_core[:, :N_DENSE].T.astype(np.float16)
    return ohdev, xdn


def run(x, w_weight, w_bias, v, trace=False, trace_kwargs=None):
    x = np.asarray(x, np.float32)
    w_weight = np.asarray(w_weight, np.float32)
    w_bias = np.asarray(w_bias, np.float32)
    v = np.asarray(v, np.float32)
    assert x.shape == (B_FULL, 39), x.shape

    vperm, vdense16 = _prep_shared(w_weight, w_bias, v)
    in_maps = []
    for i in range(N_CORES):
        xc = x[i * BC:(i + 1) * BC]
        ohdev, xdn = _prep_core(xc)
        in_maps.append({
            "oh": ohdev,
            "xdn": xdn,
            "vdense": vdense16,
            "vperm": vperm,
        })

    nc = _get_program(BC)
    res = run_bass_kernel_spmd(
        nc, in_maps, list(range(N_CORES)),
        trace=trace, **(trace_kwargs or {}),
    )
    y = np.concatenate(
        [res.results[i]["y"].T.reshape(-1, 1) for i in range(N_CORES)], axis=0
    )
    return y.astype(np.float32), res


def kernel(x, w_weight, w_bias, v):
    y, _ = run(x, w_weight, w_bias, v)
    return y


# revision 28
# speedup vs baseline: 1.0360x; 1.0360x over previous
"""FFM layer kernel for 8 Trainium2 NeuronCores — fp8 DoubleRow edition.

Math (reference): x[B,39] = 13 dense cols + 26 sparse index cols (ints 0..99
stored as f32).  inputs[B,2613] = [dense | one_hot(sparse)], then
  linear = inputs @ w.T + b
  field  = einsum('bn,nfk->bfk', inputs, v)        # [B,39,16]
  cross  = 0.5*sum_k((sum_f field)^2 - sum_f field^2)
  out    = sigmoid(linear + cross)

Strategy: data-parallel over batch, 2048 rows/core.  The sparse part is an
embedding-bag done as a one-hot matmul; the one-hot lhs is exact in fp8, so
the sparse contraction runs as fp8e4 MatmulPerfMode.DoubleRow (two 128-row
k-chunks per instruction; measured 1 cycle per output column per 256-row
pair = 2x fp16).  PSUM columns (658):
  [0:624)   field, k-major f-minor, v quantized fp8 at scale 1024 (sq only)
  [624:641) hi block: 16 U_hi cols (U = sum_f v, scale 512) + w_hi col
            (scale 1024, bias folded into the dense matmul)
  [641:658) lo block: fp8 quantization residuals at 16x the hi scale
s and linear are reconstructed as hi + lo/16, which recovers ~fp16 accuracy
for the dominant (sum_f field)^2 term; the field cols feed only sum field^2
where fp8 error is quadratically suppressed.  The first 2560 one-hot rows
fill exactly 10 DoubleRow pairs; the last 40 one-hot rows ride in the fp16
dense matmul (13 dense features + bias row + 40 one-hot rows; fp16 cost is
per-column so the extra rows are free) — this saves a whole 11th pair.

The fp8 one-hot matrix is built on the HOST and DMAed directly: at 1 byte
per cell it is the same traffic as shipping replicated indices for an
on-device is_equal build (the fp16-era tradeoff), and it frees the vector
engine for the epilogue.  One DMA per 128-row batch tile (oh_d laid out
[part, tile, chunk, col] so each transfer is one 2.5KB contiguous run per
partition) gives the PE per-tile granularity: the first matmuls start as
soon as the first 320KB tile + the first vperm pairs land.  vperm arrives
pair-by-pair for the same reason.  A short burst of warmup matmuls starts
the HAM clock ramp (cold PE runs at ~1.2GHz; full speed only arrives after
~20us of sustained activity, so the early tiles are slower regardless).
"""

import sys

sys.path.insert(0, "/opt/trn_rl_repo")

import numpy as np
import ml_dtypes

import concourse.tile as tile
from concourse import bacc, mybir
from concourse.bass_utils import run_bass_kernel_spmd

N_CORES = 8
B_FULL = 16384
BC = B_FULL // N_CORES  # 2048 rows per core
P = 128
N_DENSE = 13
N_SPARSE = 26
SPARSE_DIM = 100
N_FIELD = 39
K_DIM = 16
NOH = 20                # one-hot chunks of 128 rows in fp8 (rows 0..2559)
NPAIR = 10
NSPROWS = N_SPARSE * SPARSE_DIM  # 2600
NTAIL = NSPROWS - NOH * P        # 40 rows folded into the dense matmul
# dense-matmul stationary layout: rows 0..39 = one-hot tail, 40..63 = zeros,
# 64 = bias carrier, 65..77 = x_dense
DN0 = 64                         # first dense row
NDN = DN0 + 1 + N_DENSE          # 78 contraction rows
COLS = 658              # 624 field | 16 U_hi + w_hi | 16 U_lo + w_lo
FLD = N_FIELD * K_DIM   # 624
HI0 = FLD               # 624
LO0 = FLD + K_DIM + 1   # 641

SC_V = 1024.0
SC_U = 512.0
SC_W = 1024.0
LOGAIN = 16.0           # lo cols stored at hi_scale * LOGAIN

F16 = mybir.dt.float16
F32 = mybir.dt.float32
F8 = mybir.dt.float8e4
I8 = mybir.dt.int8

_prog_cache = {}


def _build_program(bc):
    """One SPMD program for a batch slice of `bc` rows (all cores identical)."""
    nbt = bc // P

    nc = bacc.Bacc("TRN2", target_bir_lowering=False, debug=False)
    # all dram tensors are partition-major on the host so every DMA moves
    # long contiguous runs per partition
    oh_d = nc.declare_dram_parameter(
        "oh", [P, nbt, NOH, P], F8, isOutput=False)
    xdn_d = nc.declare_dram_parameter("xdn", [NDN, bc], F16, isOutput=False)
    vd_d = nc.declare_dram_parameter("vdense", [NDN, COLS], F16,
                                     isOutput=False)
    vp_d = nc.declare_dram_parameter(
        "vperm", [P, NPAIR, 2, COLS], F8, isOutput=False)
    y_d = nc.declare_dram_parameter("y", [P, nbt], F32, isOutput=True)

    with tile.TileContext(nc) as tc:
        with (
            tc.tile_pool(name="pers", bufs=1) as pers,
            tc.tile_pool(name="psum", bufs=4, space="PSUM") as psum,
            tc.tile_pool(name="epi", bufs=3) as epi,
        ):
            oh_all = pers.tile([P, nbt, NOH, P], F8, tag="oh", name="oh")
            y_all = pers.tile([P, nbt], F32, tag="yall")
            vp_all = pers.tile([P, NPAIR, 2, COLS], F8, tag="vp")
            vd_t = pers.tile([NDN, COLS], F16, tag="vd")
            xdn_t = pers.tile([NDN, bc], F16, tag="xdn")

            # DMA plan (the head is HBM-bound: the first tile's one-hot +
            # all vperm pairs must land before tile 0 can finish):
            #   sync/scalar: alternating per-tile one-hot slices
            #   gpsimd:      warmup memsets, then vperm pair-by-pair, vdense
            #   scalar:      xdn (needed by the dense matmul, last per tile)
            wz8 = pers.tile([P, 2, P], F8, tag="wz8")
            wz8b = pers.tile([P, 2, 512], F8, tag="wz8b")
            nc.gpsimd.memset(wz8[:], 0.0)
            nc.gpsimd.memset(wz8b[:], 0.0)
            # xdn/vd lead their queues: tile 0's dense matmul (which closes
            # its PSUM accumulation group) needs them, and with 4 PSUM bufs
            # a late tile-0 close stalls the whole pipeline
            nc.scalar.dma_start(xdn_t[:, 0:512], xdn_d[:, 0:512])
            nc.gpsimd.dma_start(vd_t[:], vd_d[:])
            for bt in range(nbt):
                eng = nc.sync if bt % 2 == 0 else nc.scalar
                eng.dma_start(oh_all[:, bt, :, :], oh_d[:, bt, :, :])
            for pl, ph in ((0, 1), (1, 2), (2, 4), (4, 6), (6, 8),
                           (8, NPAIR)):
                nc.gpsimd.dma_start(vp_all[:, pl:ph, :, :],
                                    vp_d[:, pl:ph, :, :])
            nc.scalar.dma_start(xdn_t[:, 512:bc], xdn_d[:, 512:bc])

            # PE warmup: throwaway matmuls during the DMA head start the HAM
            # clock ramp before the first real matmuls are ready
            wps = psum.tile([P, COLS], F32, tag="ps", name="warmps")
            for _ in range(28):
                nc.tensor.matmul(wps[:, 0:512], wz8[:], wz8b[:],
                                 start=True, stop=True,
                                 perf_mode=mybir.MatmulPerfMode.DoubleRow)

            for bt in range(nbt):
                bs = slice(bt * P, (bt + 1) * P)
                ps = psum.tile([P, COLS], F32, tag="ps")
                # sparse one-hot contraction, fp8 DoubleRow (a single
                # 658-col matmul per pair fails the ISA check — matmul
                # output cannot cross the 2KB PSUM bank boundary, so the
                # 512-column split is mandatory)
                for p_ in range(NPAIR):
                    lhs = oh_all[:, bt, 2 * p_:2 * p_ + 2, :]
                    nc.tensor.matmul(
                        ps[:, 0:512], lhs, vp_all[:, p_, :, 0:512],
                        start=(p_ == 0), stop=False,
                        perf_mode=mybir.MatmulPerfMode.DoubleRow,
                    )
                    nc.tensor.matmul(
                        ps[:, 512:COLS], lhs, vp_all[:, p_, :, 512:COLS],
                        start=(p_ == 0), stop=False,
                        perf_mode=mybir.MatmulPerfMode.DoubleRow,
                    )
                # dense + bias + one-hot tail rows, fp16 (DoublePixel was
                # probed here and is a silent no-op on TRN2)
                nc.tensor.matmul(ps[:, 0:512], xdn_t[:, bs],
                                 vd_t[:, 0:512], start=False, stop=True)
                nc.tensor.matmul(ps[:, 512:COLS], xdn_t[:, bs],
                                 vd_t[:, 512:COLS], start=False, stop=True)
                # epilogue: s/lin = hi + lo/16; sq = sum field^2;
                # cross + sigmoid with all scale factors folded in
                t1 = epi.tile([P, K_DIM + 1], F32, tag="t1")
                nc.vector.tensor_scalar(
                    out=t1[:], in0=ps[:, LO0:COLS],
                    scalar1=1.0 / LOGAIN, scalar2=None,
                    op0=mybir.AluOpType.mult,
                )
                slin = epi.tile([P, K_DIM + 1], F32, tag="slin")
                nc.vector.tensor_tensor(
                    out=slin[:], in0=t1[:], in1=ps[:, HI0:LO0],
                    op=mybir.AluOpType.add,
                )
                if bt == nbt - 1:
                    # issue the DVE half of sq FIRST so it runs parallel
                    # with the Scalar half instead of after the s/lin chain
                    SQH = 384
                    sq_cp = epi.tile([P, FLD - SQH], F32, tag="sqcp")
                    nc.vector.tensor_copy(sq_cp[:], ps[:, SQH:FLD])
                    sq_scr2 = epi.tile([P, FLD - SQH], F32, tag="sqscr2")
                    sqsumB = epi.tile([P, 1], F32, tag="sqsumB")
                    nc.vector.scalar_tensor_tensor(
                        out=sq_scr2[:], in0=sq_cp[:],
                        scalar=1.0, in1=sq_cp[:],
                        op0=mybir.AluOpType.mult,
                        op1=mybir.AluOpType.mult,
                        accum_out=sqsumB[:],
                    )
                if bt < nbt - 1:
                    sq_scr = epi.tile([P, FLD], F32, tag="sqscr")
                    sqsum = epi.tile([P, 1], F32, tag="sqsum")
                    nc.scalar.activation(
                        out=sq_scr[:], in_=ps[:, 0:FLD],
                        func=mybir.ActivationFunctionType.Square,
                        accum_out=sqsum[:],
                    )
                else:
                    # last tile: the sq reduction is on the serial tail
                    # chain — the DVE half was issued above; Scalar does
                    # the first 384 columns in parallel
                    sq_scr = epi.tile([P, SQH], F32, tag="sqscr")
                    sqsumA = epi.tile([P, 1], F32, tag="sqsumA")
                    nc.scalar.activation(
                        out=sq_scr[:], in_=ps[:, 0:SQH],
                        func=mybir.ActivationFunctionType.Square,
                        accum_out=sqsumA[:],
                    )
                    sqsum = epi.tile([P, 1], F32, tag="sqsum")
                    nc.vector.tensor_scalar(
                        out=sqsum[:], in0=sqsumA[:],
                        scalar1=sqsumB[:], scalar2=None,
                        op0=mybir.AluOpType.add,
                    )
                s2_scr = epi.tile([P, K_DIM], F32, tag="s2scr")
                s2sum = epi.tile([P, 1], F32, tag="s2sum")
                nc.vector.scalar_tensor_tensor(
                    out=s2_scr[:], in0=slin[:, 0:K_DIM],
                    scalar=1.0, in1=slin[:, 0:K_DIM],
                    op0=mybir.AluOpType.mult,
                    op1=mybir.AluOpType.mult,
                    accum_out=s2sum[:],
                )
                # b2 = lin - 0.5*sq (true units) off the critical path
                b2a = epi.tile([P, 1], F32, tag="b2a")
                nc.vector.tensor_scalar(
                    out=b2a[:], in0=slin[:, K_DIM:K_DIM + 1],
                    scalar1=1.0 / SC_W, scalar2=None,
                    op0=mybir.AluOpType.mult,
                )
                b2 = epi.tile([P, 1], F32, tag="b2")
                nc.vector.tensor_scalar(
                    out=b2[:], in0=sqsum[:],
                    scalar1=-0.5 / (SC_V * SC_V), scalar2=b2a[:],
                    op0=mybir.AluOpType.mult,
                    op1=mybir.AluOpType.add,
                )
                nc.scalar.activation(
                    out=y_all[:, bt:bt + 1], in_=s2sum[:],
                    func=mybir.ActivationFunctionType.Sigmoid,
                    scale=0.5 / (SC_U * SC_U), bias=b2[:],
                )
            nc.sync.dma_start(y_d[:], y_all[:])

    nc.compile()
    return nc


def _get_program(bc):
    if bc not in _prog_cache:
        _prog_cache[bc] = _build_program(bc)
    return _prog_cache[bc]


def _q8(a, scale):
    return np.clip(a * scale, -240.0, 240.0).astype(ml_dtypes.float8_e4m3)


def _prep_shared(w_weight, w_bias, v):
    """vperm fp8 [128, 10, 2, 658], vdense fp16 [78, 658]."""
    # field col j = k*39 + f  <->  v[n, f, k]
    v2 = np.ascontiguousarray(v.transpose(0, 2, 1)).reshape(2613, FLD)
    u = v.sum(axis=1)                       # [2613, 16]
    w = w_weight[0]                         # [2613]

    # sparse rows 0..2559 -> fp8 hi (+ lo residual for U and w)
    ns = NOH * P
    rows = np.zeros((ns, COLS), ml_dtypes.float8_e4m3)
    sp = slice(N_DENSE, N_DENSE + ns)
    uhi = _q8(u[sp], SC_U)
    ulo = _q8(u[sp] - uhi.astype(np.float32) / SC_U, SC_U * LOGAIN)
    whi = _q8(w[sp], SC_W)
    wlo = _q8(w[sp] - whi.astype(np.float32) / SC_W, SC_W * LOGAIN)
    rows[:, 0:FLD] = _q8(v2[sp], SC_V)
    rows[:, HI0:HI0 + K_DIM] = uhi
    rows[:, HI0 + K_DIM] = whi
    rows[:, LO0:LO0 + K_DIM] = ulo
    rows[:, LO0 + K_DIM] = wlo
    # device layout [part, pair, j, col]; sparse row = (pair*2+j)*128 + part
    vperm = np.ascontiguousarray(
        rows.reshape(NPAIR, 2, P, COLS).transpose(2, 0, 1, 3))

    # dense rows at the same column scales as the fp8 side but in fp16
    tl = slice(N_DENSE + ns, 2613)
    vdense = np.zeros((NDN, COLS), np.float32)
    vdense[0:NTAIL, 0:FLD] = v2[tl] * SC_V
    vdense[0:NTAIL, HI0:HI0 + K_DIM] = u[tl] * SC_U
    vdense[0:NTAIL, HI0 + K_DIM] = w[tl] * SC_W
    vdense[DN0, HI0 + K_DIM] = float(w_bias[0]) * SC_W
    vdense[DN0 + 1:, 0:FLD] = v2[:N_DENSE] * SC_V
    vdense[DN0 + 1:, HI0:HI0 + K_DIM] = u[:N_DENSE] * SC_U
    vdense[DN0 + 1:, HI0 + K_DIM] = w[:N_DENSE] * SC_W
    return vperm, vdense.astype(np.float16)


def _prep_core(x_core):
    """Host-built fp8 one-hot [part, tile, chunk, col] and xdn fp16."""
    bc = x_core.shape[0]
    idx = x_core[:, N_DENSE:].astype(np.int32)          # [bc, 26]
    flat = idx + np.arange(N_SPARSE)[None, :] * SPARSE_DIM
    # rows 0..2559 in fp8 one-hot; rows 2560..2599 in the fp16 dense tile
    oh = np.zeros((NOH * P, bc), ml_dtypes.float8_e4m3)
    one = ml_dtypes.float8_e4m3(1.0)
    b = np.arange(bc)
    for s in range(N_SPARSE):
        r = flat[:, s]
        m = r < NOH * P
        oh[r[m], b[m]] = one
    ohdev = np.ascontiguousarray(
        oh.reshape(NOH, P, bc // P, P).transpose(1, 2, 0, 3))

    xdn = np.zeros((NDN, bc), np.float16)
    for s in range(N_SPARSE):
        r = flat[:, s]
        m = r >= NOH * P
        xdn[r[m] - NOH * P, b[m]] = 1.0
    xdn[DN0] = 1.0
    xdn[DN0 + 1:] = x_core[:, :N_DENSE].T.astype(np.float16)
    return ohdev, xdn


def run(x, w_weight, w_bias, v, trace=False, trace_kwargs=None):
    x = np.asarray(x, np.float32)
    w_weight = np.asarray(w_weight, np.float32)
    w_bias = np.asarray(w_bias, np.float32)
    v = np.asarray(v, np.float32)
    assert x.shape == (B_FULL, 39), x.shape

    vperm, vdense16 = _prep_shared(w_weight, w_bias, v)
    in_maps = []
    for i in range(N_CORES):
        xc = x[i * BC:(i + 1) * BC]
        ohdev, xdn = _prep_core(xc)
        in_maps.append({
            "oh": ohdev,
            "xdn": xdn,
            "vdense": vdense16,
            "vperm": vperm,
        })

    nc = _get_program(BC)
    res = run_bass_kernel_spmd(
        nc, in_maps, list(range(N_CORES)),
        trace=trace, **(trace_kwargs or {}),
    )
    y = np.concatenate(
        [res.results[i]["y"].T.reshape(-1, 1) for i in range(N_CORES)], axis=0
    )
    return y.astype(np.float32), res


def kernel(x, w_weight, w_bias, v):
    y, _ = run(x, w_weight, w_bias, v)
    return y
